# revision 27
# baseline (speedup 1.0000x reference)
"""Trainium2 Bass kernel for CoordLSVotingWeighted (segment_reduce).

Strategy: data-parallel over batch B=8 across 8 NeuronCores (1 image/core).
Per image, on device (pipelined over 4 sub-chunks = 2 H-halves x 2 W-slices):
  - hard one-hot of argmax over 9 seg channels (matches softmax(seg*1e6))
  - features R00 = u*y^2, m = u*x*y, R11 = u*x^2 with u = softplus(w)/(x^2+y^2)
  - segment-reduce per class via TensorE: psum[24,27] accumulates
      lhsT[pix, {hot, hot*ch, hot*cw}]^T @ rhs[pix, {R00, m, R11}]
    over 128 pixel-group matmuls.
Host: assemble 2x2 systems in float64, pinv-solve, scale by HEIGHT.

Layout: an H-half (64 rows) of each input is a single contiguous DRAM block
loaded as [128 partitions, cols]: partition = 2*(h%64) + w//64. seg is
host-reordered channel-major within each w-quarter so the one-hot compare
reads contiguously. Engine split: ACT squares/softplus/hot*ch (single act
table: exp+ln+square+copy), DVE max/one-hot/s/1s/u/features, Pool x*y/hot*cw.

Self-contained: only needs numpy / ml_dtypes / concourse (installed env).
"""

import os

import numpy as np

B = 8
H = 128
W = 128
NCLS = 9  # seg channels, class 0 = background
NPTS = 9
OC = 8
HEIGHT = 128.0
N_CORES = 8

_cache: dict = {}


def _patch_act_tables():
    """Make {Exp, Ln, Square, Copy, Identity} resolve only to the
    natural_log_exp_and_others table so the table-load pass emits exactly
    one ACT_TABLE_LOAD. Keeps every table's canonical index so emitted
    act_func_set_ids stay valid."""
    import concourse.bacc as bacc_mod
    import concourse.mybir as mybir

    if _cache.get("act_patched"):
        return
    real = bacc_mod.get_activation_tables
    F = mybir.ActivationFunctionType
    shared = {F.Exp, F.Ln, F.Square, F.Copy, F.Identity}

    def patched(arch):
        tables = real(arch)
        out = {}
        for name, funcs in tables.items():
            if name == "natural_log_exp_and_others":
                out[name] = funcs
            else:
                out[name] = funcs - shared
        return out

    bacc_mod.get_activation_tables = patched
    _cache["act_patched"] = True


def _register_recip_sum():
    """Register a fused custom DVE op: out = 1/(Src0+Src1) via the
    BITWISE_NOT exponent-flip seed + one inline Newton step (6 stages).
    ~0.4% relative error; the uniform component cancels in the host
    pinv solve. Replaces a tensor_tensor add + reciprocal_approx_fast
    pair on the bottleneck DVE stream."""
    if "recip_sum" in _cache:
        return _cache["recip_sum"]
    import re

    import numpy as np_
    import concourse.dve_ops as dve_ops
    from concourse.dve_spec import AluOp, Bin, C0, C1, Spec

    def _ref(in0, in1, c0, c1, c2):
        x = (in0 + in1).astype(np_.float32)
        not_x = (~x.view(np_.int32)).view(np_.float32)
        y0 = not_x * c0
        return y0 * (c1 - x * y0)

    _x = Bin(AluOp.ADD, dve_ops.Src0, dve_ops.Src1)
    _nx = Bin(AluOp.BITWISE_NOT, _x, _x)
    _y0 = _nx * C0
    spec = Spec(body=_y0 * (C1 - _x * _y0), reference=_ref)

    op = dve_ops.DveOp("RECIP_SUM_FAST", spec, subdim=False, uops_sha={})
    dve_ops.OPS.append(op)
    dve_ops._SUB_OPCODE_FOR_NAME[op.name] = (
        dve_ops._CUSTOM_DVE_ROW_BASE + len(dve_ops.OPS) - 1
    )
    dve_ops.CUSTOM_DVE_SPECS[op.name] = spec
    try:
        op.compile("v3")
        pinned = op
    except ValueError as e:
        sha = re.search(r"v3: (\w+) ", str(e)).group(1)
        pinned = dve_ops.DveOp(
            "RECIP_SUM_FAST", spec, subdim=False, uops_sha={"v3": sha}
        )
        dve_ops.OPS[-1] = pinned
        dve_ops.CUSTOM_DVE_SPECS[pinned.name] = spec
    _cache["recip_sum"] = pinned
    return pinned


def _build_nc():
    _patch_act_tables()
    rsum_op = _register_recip_sum()
    import concourse.bacc as bacc
    import concourse.tile as tile
    import concourse.mybir as mybir
    from concourse.alu_op_type import AluOpType as Alu

    Act = mybir.ActivationFunctionType
    Axis = mybir.AxisListType
    f32 = mybir.dt.float32
    b16 = mybir.dt.bfloat16

    nc = bacc.Bacc(
        "TRN2", target_bir_lowering=False, debug=False, num_devices=N_CORES
    )
    # One image per core; an H-half (64 rows) is contiguous in DRAM and maps
    # to 128 SBUF partitions: dram row r = 2*h + w//64  (r in [0,256)).
    seg_d = nc.dram_tensor("seg", [2 * H, W * NCLS // 2], f32, kind="ExternalInput")
    dct_d = nc.dram_tensor("direct", [2 * H, W * NPTS], f32, kind="ExternalInput")
    w_d = nc.dram_tensor("w", [2 * H, W * NPTS // 2], f32, kind="ExternalInput")
    chv_d = nc.dram_tensor("chv", [H, 2], f32, kind="ExternalInput")
    cw_d = nc.dram_tensor("cw64", [H, 64], b16, kind="ExternalInput")
    out_d = nc.dram_tensor("acc", [3 * OC, 3 * NPTS], f32, kind="ExternalOutput")

    SEGC = 576   # seg cols per half-tile   (2 q * 9 c * 32 w, channel-major)
    DCTC = 1152  # direct cols per half-tile (64 w * 9 g * 2)
    WC = 576     # w cols per half-tile      (64 w * 9 g)
    CH = 288     # feature cols per sub-chunk (32 w * 9 g)
    HC = 256     # one-hot cols per sub-chunk (8 c * 32 w)

    with tile.TileContext(nc) as tc:
        with (
            tc.tile_pool(name="main", bufs=1) as pool,
            tc.tile_pool(name="ps", bufs=1, space="PSUM") as psp,
        ):
            chunks = [(0, 0), (1, 0), (0, 1), (1, 1)]
            chv = pool.tile([H, 2], f32, tag="chv")
            cw64 = pool.tile([H, 64], b16, tag="cw64")
            seg_h = [
                pool.tile([H, SEGC], f32, name=f"seg{h}", tag=f"seg{h}")
                for h in range(2)
            ]
            dct_h = [
                pool.tile([H, DCTC], f32, name=f"dct{h}", tag=f"dct{h}")
                for h in range(2)
            ]
            w_h = [
                pool.tile([H, WC], f32, name=f"w{h}", tag=f"w{h}")
                for h in range(2)
            ]

            # DMA: consts + {w, seg} halves + direct1 on sync queue;
            # direct0 (needed early by ACT squares) on gpsimd queue.
            nc.sync.dma_start(out=chv[:, :], in_=chv_d[:, :])
            nc.sync.dma_start(out=cw64[:, :], in_=cw_d[:, :])
            nc.gpsimd.dma_start(out=dct_h[0][:, :], in_=dct_d[0:128, :])
            nc.scalar.dma_start(out=dct_h[1][:, :], in_=dct_d[128:256, :])
            nc.sync.dma_start(out=w_h[0][:, :], in_=w_d[0:128, :])
            nc.sync.dma_start(out=w_h[1][:, :], in_=w_d[128:256, :])
            nc.sync.dma_start(out=seg_h[0][:, :], in_=seg_d[0:128, :])
            nc.sync.dma_start(out=seg_h[1][:, :], in_=seg_d[128:256, :])

            acc = psp.tile([3 * OC, 3 * NPTS], f32, tag="acc")
            outs = pool.tile([3 * OC, 3 * NPTS], f32, tag="outs")
            htiles = {}
            for hf in range(2):
                htiles[hf] = dict(
                    sqx=pool.tile([H, WC], b16, name=f"sqx{hf}", tag=f"sqx{hf}"),
                    sqy=pool.tile([H, WC], b16, name=f"sqy{hf}", tag=f"sqy{hf}"),
                    ew=pool.tile([H, WC], b16, name=f"ew{hf}", tag=f"ew{hf}"),
                    sp=pool.tile([H, WC], b16, name=f"sp{hf}", tag=f"sp{hf}"),
                )
            tiles = {}
            for idx in range(4):
                tiles[idx] = dict(
                    rs=pool.tile([H, CH], b16, name=f"rs{idx}", tag=f"rs{idx}"),
                    u=pool.tile([H, CH], b16, name=f"u{idx}", tag=f"u{idx}"),
                    xy=pool.tile([H, CH], b16, name=f"xy{idx}", tag=f"xy{idx}"),
                    mx=pool.tile([H, 32], f32, name=f"mx{idx}", tag=f"mx{idx}"),
                    L=pool.tile([H, 3 * HC], b16, name=f"L{idx}", tag=f"L{idx}"),
                    R=pool.tile([H, 3 * CH], b16, name=f"R{idx}", tag=f"R{idx}"),
                )

            for idx, (hf, q) in enumerate(chunks):
                t = tiles[idx]
                ht = htiles[hf]
                seg_s = seg_h[hf][:, CH * q : CH * q + CH]
                dct_s = dct_h[hf][:, 2 * CH * q : 2 * CH * q + 2 * CH]
                dx = dct_s[:, 0::2]
                dy = dct_s[:, 1::2]
                sqx_s = ht["sqx"][:, CH * q : CH * q + CH]
                sqy_s = ht["sqy"][:, CH * q : CH * q + CH]
                sp_s = ht["sp"][:, CH * q : CH * q + CH]

                # ---- ACT: squares + softplus at half granularity
                if q == 0:
                    dx_h = dct_h[hf][:, 0::2]
                    dy_h = dct_h[hf][:, 1::2]
                    nc.scalar.activation(
                        out=ht["sqx"][:, :], in_=dx_h, func=Act.Square
                    )
                    nc.scalar.activation(
                        out=ht["sqy"][:, :], in_=dy_h, func=Act.Square
                    )
                    nc.scalar.activation(
                        out=ht["ew"][:, :], in_=w_h[hf][:, :], func=Act.Exp
                    )
                    nc.scalar.activation(
                        out=ht["sp"][:, :], in_=ht["ew"][:, :], func=Act.Ln,
                        bias=1.0,
                    )

                # ---- DVE: per-pixel max over channels (seg is channel-
                # major per quarter: col = c*32 + w) then contiguous one-hot
                seg_wc = seg_s.rearrange("p (c w) -> p w c", c=NCLS)
                nc.vector.tensor_reduce(
                    out=t["mx"][:, :], in_=seg_wc, axis=Axis.X, op=Alu.max
                )
                seg_cw = seg_s[:, 32:288].rearrange("p (c w) -> p c w", c=OC)
                mx_b = t["mx"][:, :].unsqueeze(1).broadcast_to((H, OC, 32))
                hot = t["L"][:, 0:HC].rearrange("p (c w) -> p c w", c=OC)
                nc.vector.tensor_tensor(
                    out=hot, in0=seg_cw, in1=mx_b, op=Alu.is_equal
                )
                nc.vector._custom_dve(
                    rsum_op,
                    out=t["rs"][:, :],
                    in0=sqx_s, in1=sqy_s,
                    s0=-0.23549792,
                    s1=2.0017324,
                    imm2=0.0,
                )
                nc.vector.tensor_tensor(
                    out=t["u"][:, :], in0=t["rs"][:, :], in1=sp_s,
                    op=Alu.mult,
                )

                # ---- Pool: x*y and hot*cw
                nc.gpsimd.tensor_tensor(
                    out=t["xy"][:, :], in0=dx, in1=dy, op=Alu.mult
                )

                # ---- ACT: hot * ch  (per-partition scale via Copy)
                nc.scalar.mul(
                    t["L"][:, HC : 2 * HC], t["L"][:, 0:HC], chv[:, hf : hf + 1]
                )
                hot_cw = t["L"][:, 2 * HC : 3 * HC].rearrange(
                    "p (c w) -> p c w", c=OC
                )
                cw_b = (
                    cw64[:, 32 * q : 32 * q + 32]
                    .unsqueeze(1)
                    .broadcast_to((H, OC, 32))
                )
                nc.gpsimd.tensor_tensor(
                    out=hot_cw, in0=hot, in1=cw_b, op=Alu.mult
                )

                # ---- DVE: features
                nc.vector.tensor_tensor(
                    out=t["R"][:, 0:CH], in0=t["u"][:, :], in1=sqy_s,
                    op=Alu.mult,
                )
                nc.vector.tensor_tensor(
                    out=t["R"][:, CH : 2 * CH], in0=t["u"][:, :],
                    in1=t["xy"][:, :], op=Alu.mult,
                )
                nc.vector.tensor_tensor(
                    out=t["R"][:, 2 * CH : 3 * CH], in0=t["u"][:, :],
                    in1=sqx_s, op=Alu.mult,
                )

                # ---- PE: 32 accumulating matmuls (one per w-subcolumn)
                Lv = t["L"][:, :].rearrange("p (t w) -> p t w", w=32)
                Rv = t["R"][:, :].rearrange("p (b n) -> p b n", b=3)
                for j in range(32):
                    nc.tensor.matmul(
                        acc[:, :],
                        Lv[:, :, j],
                        Rv[:, :, NPTS * j : NPTS * j + NPTS],
                        start=(idx == 0 and j == 0),
                        stop=(idx == 3 and j == 31),
                    )

            nc.scalar.copy(out=outs[:, :], in_=acc[:, :])
            nc.sync.dma_start(out=out_d[:, :], in_=outs[:, :])

    nc.compile()
    return nc


def _host_constants():
    import ml_dtypes

    bf16 = ml_dtypes.bfloat16
    pi = np.arange(128)
    chv = np.stack(
        [(64.0 * hf + pi // 2 + 0.5) / HEIGHT for hf in range(2)], axis=1
    ).astype(np.float32)
    j = np.arange(64)
    cw64 = (((pi % 2)[:, None] * 64 + j[None, :] + 0.5) / HEIGHT).astype(bf16)
    return chv, np.ascontiguousarray(cw64)


def _solve_host(acc_f32: np.ndarray) -> np.ndarray:
    """acc [24,27] fp32 -> p [OC, NPTS, 2] fp32 (float64 pinv like reference)."""
    a = acc_f32.astype(np.float64)
    A = a[0:OC, 0:9]
    Bm = a[0:OC, 9:18]
    D = a[0:OC, 18:27]
    S1 = a[OC : 2 * OC, 0:9]
    S3 = a[OC : 2 * OC, 9:18]
    S2 = a[2 * OC : 3 * OC, 9:18]
    S4 = a[2 * OC : 3 * OC, 18:27]
    Rm = np.empty((OC, NPTS, 2, 2), dtype=np.float64)
    Rm[..., 0, 0] = A
    Rm[..., 0, 1] = -Bm
    Rm[..., 1, 0] = -Bm
    Rm[..., 1, 1] = D
    q = np.stack([S1 - S2, S4 - S3], axis=-1)
    Rp = np.linalg.pinv(Rm.reshape(-1, 2, 2)).reshape(Rm.shape)
    p = np.einsum("cpij,cpj->cpi", Rp, q) * HEIGHT
    return p.astype(np.float32)


def kernel(seg, direct, w):
    if "nc" not in _cache:
        _cache["nc"] = _build_nc()
    nc = _cache["nc"]

    seg = np.ascontiguousarray(np.asarray(seg, dtype=np.float32))
    direct = np.ascontiguousarray(np.asarray(direct, dtype=np.float32))
    w = np.ascontiguousarray(np.asarray(w, dtype=np.float32))
    chv, cw64 = _host_constants()

    in_maps = []
    for i in range(B):
        in_maps.append(
            {
                "seg": np.ascontiguousarray(
                    seg[i].reshape(128, 2, 2, 32, NCLS).transpose(0, 1, 2, 4, 3)
                ).reshape(2 * H, W * NCLS // 2),
                "direct": direct[i].reshape(2 * H, W * NPTS),
                "w": w[i].reshape(2 * H, W * NPTS // 2),
                "chv": chv,
                "cw64": cw64,
            }
        )

    from concourse.bass_utils import run_bass_kernel_spmd

    trace = bool(int(os.environ.get("KERNEL_TRACE", "0")))
    res = run_bass_kernel_spmd(
        nc, in_maps, core_ids=list(range(N_CORES)), trace=trace
    )
    kernel._last_exec_ns = res.exec_time_ns
    kernel._last_results = res

    out = np.stack(
        [_solve_host(np.asarray(res.results[i]["acc"])) for i in range(B)], axis=0
    )
    return out


# revision 28
# speedup vs baseline: 1.0667x; 1.0667x over previous
"""Trainium2 Bass kernel for CoordLSVotingWeighted (segment_reduce).

Strategy: data-parallel over batch B=8 across 8 NeuronCores (1 image/core).
Per image, on device (pipelined over 4 sub-chunks = 2 H-halves x 2 W-slices):
  - hard one-hot of argmax over 9 seg channels (matches softmax(seg*1e6))
  - features R00 = u*y^2, m = u*x*y, R11 = u*x^2 with u = softplus(w)/(x^2+y^2)
  - segment-reduce per class via TensorE: psum[24,27] accumulates
      lhsT[pix, {hot, hot*ch, hot*cw}]^T @ rhs[pix, {R00, m, R11}]
    over 128 pixel-group matmuls.
Host: assemble 2x2 systems in float64, pinv-solve, scale by HEIGHT.

Layout: an H-half (64 rows) of each input is a single contiguous DRAM block
loaded as [128 partitions, cols]: partition = 2*(h%64) + w//64. seg is
host-reordered channel-major within each w-quarter so the one-hot compare
reads contiguously. Engine split: ACT squares/softplus/hot*ch (single act
table: exp+ln+square+copy), DVE max/one-hot/s/1s/u/features, Pool x*y/hot*cw.

Self-contained: only needs numpy / ml_dtypes / concourse (installed env).
"""

import os

import numpy as np

B = 8
H = 128
W = 128
NCLS = 9  # seg channels, class 0 = background
NPTS = 9
OC = 8
HEIGHT = 128.0
N_CORES = 8

_cache: dict = {}


def _patch_act_tables():
    """Make {Exp, Ln, Square, Copy, Identity} resolve only to the
    natural_log_exp_and_others table so the table-load pass emits exactly
    one ACT_TABLE_LOAD. Keeps every table's canonical index so emitted
    act_func_set_ids stay valid."""
    import concourse.bacc as bacc_mod
    import concourse.mybir as mybir

    if _cache.get("act_patched"):
        return
    real = bacc_mod.get_activation_tables
    F = mybir.ActivationFunctionType
    shared = {F.Exp, F.Ln, F.Square, F.Copy, F.Identity}

    def patched(arch):
        tables = real(arch)
        out = {}
        for name, funcs in tables.items():
            if name == "natural_log_exp_and_others":
                out[name] = funcs
            else:
                out[name] = funcs - shared
        return out

    bacc_mod.get_activation_tables = patched
    _cache["act_patched"] = True


def _register_recip_sum():
    """Register a fused custom DVE op: out = 1/(Src0+Src1) via the
    BITWISE_NOT exponent-flip seed + one inline Newton step (6 stages).
    ~0.4% relative error; the uniform component cancels in the host
    pinv solve. Replaces a tensor_tensor add + reciprocal_approx_fast
    pair on the bottleneck DVE stream."""
    if "recip_sum" in _cache:
        return _cache["recip_sum"]
    import re

    import numpy as np_
    import concourse.dve_ops as dve_ops
    from concourse.dve_spec import AluOp, Bin, C0, C1, Spec

    def _ref(in0, in1, c0, c1, c2):
        x = (in0 + in1).astype(np_.float32)
        not_x = (~x.view(np_.int32)).view(np_.float32)
        y0 = not_x * c0
        return y0 * (c1 - x * y0)

    _x = Bin(AluOp.ADD, dve_ops.Src0, dve_ops.Src1)
    _nx = Bin(AluOp.BITWISE_NOT, _x, _x)
    _y0 = _nx * C0
    spec = Spec(body=_y0 * (C1 - _x * _y0), reference=_ref)

    op = dve_ops.DveOp("RECIP_SUM_FAST", spec, subdim=False, uops_sha={})
    dve_ops.OPS.append(op)
    dve_ops._SUB_OPCODE_FOR_NAME[op.name] = (
        dve_ops._CUSTOM_DVE_ROW_BASE + len(dve_ops.OPS) - 1
    )
    dve_ops.CUSTOM_DVE_SPECS[op.name] = spec
    try:
        op.compile("v3")
        pinned = op
    except ValueError as e:
        sha = re.search(r"v3: (\w+) ", str(e)).group(1)
        pinned = dve_ops.DveOp(
            "RECIP_SUM_FAST", spec, subdim=False, uops_sha={"v3": sha}
        )
        dve_ops.OPS[-1] = pinned
        dve_ops.CUSTOM_DVE_SPECS[pinned.name] = spec
    _cache["recip_sum"] = pinned
    return pinned


def _build_nc():
    _patch_act_tables()
    rsum_op = _register_recip_sum()
    import concourse.bacc as bacc
    import concourse.tile as tile
    import concourse.mybir as mybir
    from concourse.alu_op_type import AluOpType as Alu

    Act = mybir.ActivationFunctionType
    Axis = mybir.AxisListType
    f32 = mybir.dt.float32
    b16 = mybir.dt.bfloat16

    nc = bacc.Bacc(
        "TRN2", target_bir_lowering=False, debug=False, num_devices=N_CORES
    )
    # One image per core; an H-half (64 rows) is contiguous in DRAM and maps
    # to 128 SBUF partitions: dram row r = 2*h + w//64  (r in [0,256)).
    seg_d = nc.dram_tensor("seg", [2 * H, W * NCLS // 2], f32, kind="ExternalInput")
    dct_d = nc.dram_tensor("direct", [2 * H, W * NPTS], f32, kind="ExternalInput")
    w_d = nc.dram_tensor("w", [2 * H, W * NPTS // 2], f32, kind="ExternalInput")
    chv_d = nc.dram_tensor("chv", [H, 2], f32, kind="ExternalInput")
    cw_d = nc.dram_tensor("cw64", [H, 64], b16, kind="ExternalInput")
    out_d = nc.dram_tensor("acc", [3 * OC, 3 * NPTS], f32, kind="ExternalOutput")

    SEGC = 576   # seg cols per half-tile   (2 q * 9 c * 32 w, channel-major)
    DCTC = 1152  # direct cols per half-tile (64 w * 9 g * 2)
    WC = 576     # w cols per half-tile      (64 w * 9 g)
    CH = 288     # feature cols per sub-chunk (32 w * 9 g)
    HC = 256     # one-hot cols per sub-chunk (8 c * 32 w)

    with tile.TileContext(nc) as tc:
        with (
            tc.tile_pool(name="main", bufs=1) as pool,
            tc.tile_pool(name="ps", bufs=1, space="PSUM") as psp,
        ):
            chunks = [(0, 0), (0, 1), (1, 0), (1, 1)]
            chv = pool.tile([H, 2], f32, tag="chv")
            cw64 = pool.tile([H, 64], b16, tag="cw64")
            seg_h = [
                pool.tile([H, SEGC], f32, name=f"seg{h}", tag=f"seg{h}")
                for h in range(2)
            ]
            dct_h = [
                pool.tile([H, DCTC], f32, name=f"dct{h}", tag=f"dct{h}")
                for h in range(2)
            ]
            w_h = [
                pool.tile([H, WC], f32, name=f"w{h}", tag=f"w{h}")
                for h in range(2)
            ]

            # DMA: consts + {w, seg} halves + direct1 on sync queue;
            # direct0 (needed early by ACT squares) on gpsimd queue.
            nc.sync.dma_start(out=chv[:, :], in_=chv_d[:, :])
            nc.sync.dma_start(out=cw64[:, :], in_=cw_d[:, :])
            nc.gpsimd.dma_start(out=dct_h[0][:, :], in_=dct_d[0:128, :])
            nc.scalar.dma_start(out=dct_h[1][:, :], in_=dct_d[128:256, :])
            nc.sync.dma_start(out=w_h[0][:, :], in_=w_d[0:128, :])
            nc.sync.dma_start(out=seg_h[0][:, :], in_=seg_d[0:128, :])
            nc.sync.dma_start(out=w_h[1][:, :], in_=w_d[128:256, :])
            nc.sync.dma_start(out=seg_h[1][:, :], in_=seg_d[128:256, :])

            acc = psp.tile([3 * OC, 3 * NPTS], f32, tag="acc")
            outs = pool.tile([3 * OC, 3 * NPTS], f32, tag="outs")
            htiles = {}
            for hf in range(2):
                htiles[hf] = dict(
                    sqx=pool.tile([H, WC], b16, name=f"sqx{hf}", tag=f"sqx{hf}"),
                    sqy=pool.tile([H, WC], b16, name=f"sqy{hf}", tag=f"sqy{hf}"),
                    ew=pool.tile([H, WC], b16, name=f"ew{hf}", tag=f"ew{hf}"),
                    sp=pool.tile([H, WC], b16, name=f"sp{hf}", tag=f"sp{hf}"),
                )
            tiles = {}
            for idx in range(4):
                tiles[idx] = dict(
                    rs=pool.tile([H, CH], b16, name=f"rs{idx}", tag=f"rs{idx}"),
                    u=pool.tile([H, CH], b16, name=f"u{idx}", tag=f"u{idx}"),
                    xy=pool.tile([H, CH], b16, name=f"xy{idx}", tag=f"xy{idx}"),
                    mx=pool.tile([H, 32], f32, name=f"mx{idx}", tag=f"mx{idx}"),
                    L=pool.tile([H, 3 * HC], b16, name=f"L{idx}", tag=f"L{idx}"),
                    R=pool.tile([H, 3 * CH], b16, name=f"R{idx}", tag=f"R{idx}"),
                )

            for idx, (hf, q) in enumerate(chunks):
                t = tiles[idx]
                ht = htiles[hf]
                seg_s = seg_h[hf][:, CH * q : CH * q + CH]
                dct_s = dct_h[hf][:, 2 * CH * q : 2 * CH * q + 2 * CH]
                dx = dct_s[:, 0::2]
                dy = dct_s[:, 1::2]
                sqx_s = ht["sqx"][:, CH * q : CH * q + CH]
                sqy_s = ht["sqy"][:, CH * q : CH * q + CH]
                sp_s = ht["sp"][:, CH * q : CH * q + CH]

                # ---- ACT: squares + softplus at half granularity
                if q == 0:
                    dx_h = dct_h[hf][:, 0::2]
                    dy_h = dct_h[hf][:, 1::2]
                    nc.scalar.activation(
                        out=ht["sqx"][:, :], in_=dx_h, func=Act.Square
                    )
                    nc.scalar.activation(
                        out=ht["sqy"][:, :], in_=dy_h, func=Act.Square
                    )
                    nc.scalar.activation(
                        out=ht["ew"][:, :], in_=w_h[hf][:, :], func=Act.Exp
                    )
                    nc.scalar.activation(
                        out=ht["sp"][:, :], in_=ht["ew"][:, :], func=Act.Ln,
                        bias=1.0,
                    )

                # ---- DVE: per-pixel max over channels (seg is channel-
                # major per quarter: col = c*32 + w) then contiguous one-hot
                seg_wc = seg_s.rearrange("p (c w) -> p w c", c=NCLS)
                nc.vector.tensor_reduce(
                    out=t["mx"][:, :], in_=seg_wc, axis=Axis.X, op=Alu.max
                )
                seg_cw = seg_s[:, 32:288].rearrange("p (c w) -> p c w", c=OC)
                mx_b = t["mx"][:, :].unsqueeze(1).broadcast_to((H, OC, 32))
                hot = t["L"][:, 0:HC].rearrange("p (c w) -> p c w", c=OC)
                nc.vector.tensor_tensor(
                    out=hot, in0=seg_cw, in1=mx_b, op=Alu.is_equal
                )
                nc.vector._custom_dve(
                    rsum_op,
                    out=t["rs"][:, :],
                    in0=sqx_s, in1=sqy_s,
                    s0=-0.23549792,
                    s1=2.0017324,
                    imm2=0.0,
                )
                nc.vector.tensor_tensor(
                    out=t["u"][:, :], in0=t["rs"][:, :], in1=sp_s,
                    op=Alu.mult,
                )

                # ---- Pool: x*y and hot*cw
                nc.gpsimd.tensor_tensor(
                    out=t["xy"][:, :], in0=dx, in1=dy, op=Alu.mult
                )

                # ---- ACT: hot * ch  (per-partition scale via Copy)
                nc.scalar.mul(
                    t["L"][:, HC : 2 * HC], t["L"][:, 0:HC], chv[:, hf : hf + 1]
                )
                hot_cw = t["L"][:, 2 * HC : 3 * HC].rearrange(
                    "p (c w) -> p c w", c=OC
                )
                cw_b = (
                    cw64[:, 32 * q : 32 * q + 32]
                    .unsqueeze(1)
                    .broadcast_to((H, OC, 32))
                )
                nc.gpsimd.tensor_tensor(
                    out=hot_cw, in0=hot, in1=cw_b, op=Alu.mult
                )

                # ---- DVE: features
                nc.vector.tensor_tensor(
                    out=t["R"][:, 0:CH], in0=t["u"][:, :], in1=sqy_s,
                    op=Alu.mult,
                )
                nc.vector.tensor_tensor(
                    out=t["R"][:, CH : 2 * CH], in0=t["u"][:, :],
                    in1=t["xy"][:, :], op=Alu.mult,
                )
                nc.vector.tensor_tensor(
                    out=t["R"][:, 2 * CH : 3 * CH], in0=t["u"][:, :],
                    in1=sqx_s, op=Alu.mult,
                )

                # ---- PE: 32 accumulating matmuls (one per w-subcolumn)
                Lv = t["L"][:, :].rearrange("p (t w) -> p t w", w=32)
                Rv = t["R"][:, :].rearrange("p (b n) -> p b n", b=3)
                for j in range(32):
                    nc.tensor.matmul(
                        acc[:, :],
                        Lv[:, :, j],
                        Rv[:, :, NPTS * j : NPTS * j + NPTS],
                        start=(idx == 0 and j == 0),
                        stop=(idx == 3 and j == 31),
                    )

            nc.scalar.copy(out=outs[:, :], in_=acc[:, :])
            nc.sync.dma_start(out=out_d[:, :], in_=outs[:, :])

    nc.compile()
    return nc


def _host_constants():
    import ml_dtypes

    bf16 = ml_dtypes.bfloat16
    pi = np.arange(128)
    chv = np.stack(
        [(64.0 * hf + pi // 2 + 0.5) / HEIGHT for hf in range(2)], axis=1
    ).astype(np.float32)
    j = np.arange(64)
    cw64 = (((pi % 2)[:, None] * 64 + j[None, :] + 0.5) / HEIGHT).astype(bf16)
    return chv, np.ascontiguousarray(cw64)


def _solve_host(acc_f32: np.ndarray) -> np.ndarray:
    """acc [24,27] fp32 -> p [OC, NPTS, 2] fp32 (float64 pinv like reference)."""
    a = acc_f32.astype(np.float64)
    A = a[0:OC, 0:9]
    Bm = a[0:OC, 9:18]
    D = a[0:OC, 18:27]
    S1 = a[OC : 2 * OC, 0:9]
    S3 = a[OC : 2 * OC, 9:18]
    S2 = a[2 * OC : 3 * OC, 9:18]
    S4 = a[2 * OC : 3 * OC, 18:27]
    Rm = np.empty((OC, NPTS, 2, 2), dtype=np.float64)
    Rm[..., 0, 0] = A
    Rm[..., 0, 1] = -Bm
    Rm[..., 1, 0] = -Bm
    Rm[..., 1, 1] = D
    q = np.stack([S1 - S2, S4 - S3], axis=-1)
    Rp = np.linalg.pinv(Rm.reshape(-1, 2, 2)).reshape(Rm.shape)
    p = np.einsum("cpij,cpj->cpi", Rp, q) * HEIGHT
    return p.astype(np.float32)


def kernel(seg, direct, w):
    if "nc" not in _cache:
        _cache["nc"] = _build_nc()
    nc = _cache["nc"]

    seg = np.ascontiguousarray(np.asarray(seg, dtype=np.float32))
    direct = np.ascontiguousarray(np.asarray(direct, dtype=np.float32))
    w = np.ascontiguousarray(np.asarray(w, dtype=np.float32))
    chv, cw64 = _host_constants()

    in_maps = []
    for i in range(B):
        in_maps.append(
            {
                "seg": np.ascontiguousarray(
                    seg[i].reshape(128, 2, 2, 32, NCLS).transpose(0, 1, 2, 4, 3)
                ).reshape(2 * H, W * NCLS // 2),
                "direct": direct[i].reshape(2 * H, W * NPTS),
                "w": w[i].reshape(2 * H, W * NPTS // 2),
                "chv": chv,
                "cw64": cw64,
            }
        )

    from concourse.bass_utils import run_bass_kernel_spmd

    trace = bool(int(os.environ.get("KERNEL_TRACE", "0")))
    res = run_bass_kernel_spmd(
        nc, in_maps, core_ids=list(range(N_CORES)), trace=trace
    )
    kernel._last_exec_ns = res.exec_time_ns
    kernel._last_results = res

    out = np.stack(
        [_solve_host(np.asarray(res.results[i]["acc"])) for i in range(B)], axis=0
    )
    return out
